# revision 1
# baseline (speedup 1.0000x reference)
"""Trainium2 Bass kernel for causal self-attention (B=4, T=2048, C=1024, H=16).

Sharding: 2 heads per core across 8 cores (tensor parallel on heads).
Per core, all operands fp16 (values O(1); |S*scale| < ~6 so exp needs no
max-subtraction), per batch:
  1. QKV projection for its 128 channels, q/k/v kept transposed [ch, tok]
     in SBUF; x chunks for the NEXT batch are prefetched before this
     batch's attention so they never queue behind attention-phase DMAs.
  2. Causal attention per head: scores computed TRANSPOSED (S^T [s, t]) so
     the softmax denominator comes out of the same matmul that applies V:
     lhsT = [v_h | ones] makes PSUM rows 64:128 the row-sum Z. V is
     transposed into [s, ch] via PE identity-matmuls; both heads' S
     matmuls are emitted back-to-back (disjoint PE row groups).
  3. One AllToAll per batch ([128ch, 256tok] per dest) fires right after
     the batch; each core then projects its 256 tokens with a RESIDENT
     full Wp (2MB SBUF), deferred two batches so the yg loads never park
     at the DMA queue head behind a still-running collective.  The last
     batch uses two half-size AllToAlls so only a ~22us collective and a
     128-token projection remain in the tail.
Host side: x pre-transposed, weights pre-packed [128, ck, out] for
single-DMA loads; output slices are concatenated and bp added at the end.
"""

import numpy as np

import concourse.bass as bass
import concourse.mybir as mybir
import concourse.tile as tile
from concourse import bacc

F32 = mybir.dt.float32
F32R = mybir.dt.float32r
F16 = mybir.dt.float16
EXP = mybir.ActivationFunctionType.Exp

# problem shape (hardcoded per harness contract)
B, T, C, H = 4, 2048, 1024, 16
D = C // H              # 64
NCORES = 8
BT = B * T
TSL = BT // NCORES      # tokens per core after AllToAll
SCALE = 1.0 / np.sqrt(np.float32(D))


def build_program(b=B, t=T, c=C, ncores=NCORES, reps=1):
    """Build the SPMD single-core program. Requires c == 128 * ncores."""
    assert c == 128 * ncores, "2 heads of 64 dims per core"
    bt = b * t
    tsl = bt // ncores
    nk = c // 128            # contraction tiles for projections
    tch = t // 512           # 512-token chunks per batch
    sbk = t // 128           # 128-token s-blocks per batch

    hcho = t // 256 // tch        # half-chunks per 512-chunk (2)
    nc = bacc.Bacc("TRN2", target_bir_lowering=False, num_devices=ncores)

    # host-packed weights: [partition, ck, out-ch] so each loads in ONE DMA
    # with 2KB+ contiguous runs per partition (HWDGE overhead is per-DMA)
    xT = nc.dram_tensor("xT", [c, bt], F16, kind="ExternalInput")
    wqP = nc.dram_tensor("wqP", [128, nk, 128], F16, kind="ExternalInput")
    wkP = nc.dram_tensor("wkP", [128, nk, 128], F16, kind="ExternalInput")
    wvP = nc.dram_tensor("wvP", [128, nk, 128], F16, kind="ExternalInput")
    bq = nc.dram_tensor("bq", [128, 1], F32, kind="ExternalInput")
    bk = nc.dram_tensor("bk", [128, 1], F32, kind="ExternalInput")
    bv = nc.dram_tensor("bv", [128, 1], F32, kind="ExternalInput")
    wpP = nc.dram_tensor("wpP", [128, nk, c], F16, kind="ExternalInput")
    ident = nc.dram_tensor("ident", [128, 128], F16, kind="ExternalInput")
    outT = nc.dram_tensor("outT", [c, tsl], F32, kind="ExternalOutput")

    with tile.TileContext(nc) as tc:
        with (
            tc.tile_pool(name="singles", bufs=1) as singles,
            tc.tile_pool(name="dram", bufs=1, space="DRAM") as dram,
            tc.tile_pool(name="xin", bufs=8) as xin,
            tc.tile_pool(name="qkv", bufs=2) as qkv,
            tc.tile_pool(name="vva", bufs=2) as vva,
            tc.tile_pool(name="ptile", bufs=4) as ptile,
            tc.tile_pool(name="ynorm", bufs=3) as ynorm,
            tc.tile_pool(name="wp", bufs=1) as wppool,
            tc.tile_pool(name="outsb", bufs=2) as outsb,
            tc.tile_pool(name="yg", bufs=2) as ygpool,
            tc.tile_pool(name="ps_s", bufs=2, space="PSUM") as ps_s,
            tc.tile_pool(name="ps_y", bufs=1, space="PSUM") as ps_y,
            tc.tile_pool(name="ps_mm", bufs=2, space="PSUM") as ps_mm,
        ):
            # one AllToAll per batch: each core sends [128ch, 256tok] to each
            # dest; core d ends up with batch tokens [256d, 256d+256) full-C.
            # The LAST batch uses two half-batch AllToAlls ([128ch, 128tok]
            # slices) so the first fires mid-attention and only a small
            # collective remains in the tail.
            a2a_ins = [dram.tile([ncores, 128, 256], F16, name=f"a2ai{i}")
                       for i in range(b - 1)]
            a2a_outs = [dram.tile([ncores, 128, 256], F16, name=f"a2ao{i}")
                        for i in range(b - 1)]
            a2a_ins_h = [dram.tile([ncores, 128, 128], F16, name=f"a2aih{i}")
                         for i in range(2)]
            a2a_outs_h = [dram.tile([ncores, 128, 128], F16, name=f"a2aoh{i}")
                          for i in range(2)]

            # --- constants (wq first: the first matmuls need only wq + x;
            # the rest of the singles load AFTER the first x chunk so they
            # don't delay it on the serial DMA-generation path) ---
            w_all = {}
            bias_tiles = {}

            def load_wb(nm, wt, bias, split=False):
                wtile = singles.tile([128, nk, 128], F16, name=f"w{nm}")
                if split:
                    # first ck tile lands alone so the very first projection
                    # matmul isn't gated on the full weight pack
                    nc.sync.dma_start(out=wtile[:, 0:1, :], in_=wt[:, 0:1, :])
                    nc.sync.dma_start(out=wtile[:, 1:nk, :], in_=wt[:, 1:nk, :])
                else:
                    nc.sync.dma_start(out=wtile, in_=wt[:, :, :])
                w_all[nm] = wtile
                btile = singles.tile([128, 1], F32, name=f"b{nm}")
                nc.sync.dma_start(out=btile, in_=bias[:, :])
                bias_tiles[nm] = btile

            load_wb("q", wqP, bq, split=True)
            identity = singles.tile([128, 128], F16)
            cmask = singles.tile([128, 128], F32)
            nc.gpsimd.memset(cmask, 0.0)
            # keep (0) where t - s >= 0 else -1e10
            nc.gpsimd.affine_select(
                out=cmask, in_=cmask, compare_op=mybir.AluOpType.is_ge,
                fill=-1e10, base=0, channel_multiplier=-1, pattern=[[1, 128]],
            )

            def load_rest_singles():
                load_wb("k", wkP, bk)
                load_wb("v", wvP, bv)
                nc.sync.dma_start(out=identity, in_=ident[:, :])



            wpr_holder = {}

            def out_proj(bis):
                # out-proj over one or more batches' 256-token slices in a
                # single N=256*len(bis) pass; outT stores split in halves so
                # the first store overlaps the second half's matmuls
                wpr = wpr_holder["wp"]
                nb = len(bis)
                wtok = 256 * nb
                hk = nk // 2
                ygall = ygpool.tile([128, nk, wtok], F16, tag="yg", name="yg")
                for u, bi in enumerate(bis):
                    nc.sync.dma_start(
                        out=ygall[:, :, 256 * u:256 * u + 256],
                        in_=a2a_outs[bi].rearrange("s p g -> p s g"))
                osball = outsb.tile([128, nk, wtok], F32, tag="osb",
                                    name="osb")
                for ph in range(2):
                    for ot in range(hk * ph, hk * ph + hk):
                        ops = ps_mm.tile([128, wtok], F32, tag="mm",
                                         name="ops")
                        for ck in range(nk):
                            nc.tensor.matmul(
                                ops, wpr[:, ck, 128 * ot:128 * ot + 128],
                                ygall[:, ck, :],
                                start=(ck == 0), stop=(ck == nk - 1))
                        nc.vector.tensor_copy(osball[:, ot, :], ops)
                    nc.sync.dma_start(
                        out=outT.rearrange("(ot p) g -> p ot g", p=128)[
                            :, hk * ph:hk * ph + hk,
                            256 * bis[0]:256 * bis[0] + wtok],
                        in_=osball[:, hk * ph:hk * ph + hk, :])

            def load_x(bi, j, split=1):
                t0 = bi * t
                xt = xin.tile([128, nk, 512], F16, tag="xt", name="xt")
                xv = xT.rearrange("(ck p) g -> p ck g", p=128)
                step = nk // split
                for u in range(split):
                    nc.sync.dma_start(
                        out=xt[:, step * u:step * u + step, :],
                        in_=xv[:, step * u:step * u + step,
                               t0 + 512 * j:t0 + 512 * j + 512],
                    )
                return xt

            def out_proj_h(hb):
                # half-batch (128-token) out-proj for the last batch
                wpr = wpr_holder["wp"]
                ygall = ygpool.tile([128, nk, 128], F16, tag="ygh",
                                    name="ygh")
                nc.sync.dma_start(
                    out=ygall,
                    in_=a2a_outs_h[hb].rearrange("s p g -> p s g"))
                osball = outsb.tile([128, nk, 128], F32, tag="osbh",
                                    name="osbh")
                hk = nk // 2
                for ph in range(2):
                    for ot in range(hk * ph, hk * ph + hk):
                        ops = ps_mm.tile([128, 128], F32, tag="mm",
                                         name="ops")
                        for ck in range(nk):
                            nc.tensor.matmul(
                                ops, wpr[:, ck, 128 * ot:128 * ot + 128],
                                ygall[:, ck, :],
                                start=(ck == 0), stop=(ck == nk - 1))
                        nc.vector.tensor_copy(osball[:, ot, :], ops)
                    nc.sync.dma_start(
                        out=outT.rearrange("(ot p) g -> p ot g", p=128)[
                            :, hk * ph:hk * ph + hk,
                            256 * (b - 1) + 128 * hb:
                            256 * (b - 1) + 128 * hb + 128],
                        in_=osball[:, hk * ph:hk * ph + hk, :])

            # --- per batch: projection, v-prep, attention ---
            xts = None
            for _rep in range(reps):
              wpr = None
              if xts is None:
                  # batch-0 chunk 0 first (split finely so the very first
                  # projection matmul starts as early as possible), THEN the
                  # remaining weights, then the other chunks
                  xts = [load_x(0, 0, split=2)]
                  load_rest_singles()
                  xts += [load_x(0, j) for j in range(1, tch)]
              for bi in range(b):
                  qT = qkv.tile([128, t], F16, tag="qT")
                  kT = qkv.tile([128, t], F16, tag="kT")
                  vT = qkv.tile([128, t], F16, tag="vT")
                  for j in range(tch):
                      xt = xts[j]
                      for nm, dst in (("q", qT), ("k", kT), ("v", vT)):
                          ps = ps_mm.tile([128, 512], F32, tag="mm")
                          for ck in range(nk):
                              nc.tensor.matmul(
                                  ps, w_all[nm][:, ck, :], xt[:, ck, :],
                                  start=(ck == 0), stop=(ck == nk - 1),
                              )
                          nc.vector.tensor_scalar_add(
                              dst[:, 512 * j:512 * j + 512], ps, bias_tiles[nm])
                  # prefetch the next batch's x now (or the next rep's batch
                  # 0): these DMAs enqueue ahead of this batch's yt stores,
                  # so the next proj phase never starves behind
                  # end-of-attention queue traffic
                  if bi + 1 < b:
                      xts = [load_x(bi + 1, j) for j in range(tch)]
                  elif _rep + 1 < reps:
                      xts = [load_x(0, j) for j in range(tch)]

                  # v -> [s, ch] as [v_h0 | ones | v_h1 | ones]: matmul lhsT
                  # slices stay contiguous (BIR wants one free dim) while the
                  # PSUM->SBUF copy is a single strided op per s-block
                  vv = vva.tile([128, sbk, 4, 64], F16, tag="vv")
                  nc.vector.memset(vv[:, :, 1, :], 1.0)
                  nc.vector.memset(vv[:, :, 3, :], 1.0)
                  for i in range(sbk):
                      vps = ps_mm.tile([128, 128], F16, tag="mm")
                      nc.tensor.transpose(
                          vps, vT[:, 128 * i:128 * i + 128], identity)
                      nc.vector.tensor_copy(
                          vv[:, i, 0::2, :],
                          vps.rearrange("p (s g) -> p s g", s=2))

                  # previous rep's tail out-projections emitted AFTER this
                  # rep's batch-0 proj AND v-prep: both sit ahead of them in
                  # the in-order PE queue / PSUM pool rotation, so the PE
                  # fills the previous rep's final-collective window instead
                  # of stalling behind it
                  if bi == 0 and _rep > 0:
                      out_proj_h(0)
                      out_proj_h(1)

                  for j in range(tch):
                      yps = []
                      for h in range(2):
                          yp = ps_y.tile([128, 512], F32, tag=f"yp{h}")
                          yps.append(yp)
                      nsb = 4 * j + 4
                      for i in range(nsb):
                          toff = max(0, 128 * i - 512 * j)
                          w = 512 - toff
                          # S matmuls for both heads emitted back-to-back:
                          # they hit disjoint PE row groups (partitions 0-63
                          # vs 64-127) so hardware can overlap them
                          sps = []
                          for h in range(2):
                              d0 = 64 * h
                              sp = ps_s.tile([128, 512], F32, tag=f"sp{h}")
                              nc.tensor.matmul(
                                  sp[:, :w],
                                  kT[d0:d0 + 64, 128 * i:128 * i + 128],
                                  qT[d0:d0 + 64, 512 * j + toff:512 * j + 512],
                                  start=True, stop=True,
                              )
                              sps.append(sp)
                          for h in range(2):
                              sp = sps[h]
                              if 128 * i >= 512 * j:
                                  nc.vector.tensor_add(
                                      sp[:, 0:128], sp[:, 0:128], cmask)
                              pt = ptile.tile([128, 512], F16, tag=f"p{h}")
                              nc.scalar.activation(
                                  pt[:, :w], sp[:, :w], EXP, scale=float(SCALE))
                              nc.tensor.matmul(
                                  yps[h][:, toff:512],
                                  vv[:, i, 2 * h:2 * h + 2, :],
                                  pt[:, :w],
                                  start=(i == 0), stop=(i == nsb - 1),
                                  skip_group_check=True,
                              )
                      for h in range(2):
                          zr = ynorm.tile([64, 512], F32, tag="zr")
                          nc.vector.reciprocal(zr, yps[h][64:128, :])
                          if bi < b - 1:
                              yt = ynorm.tile([64, 512], F16, tag="yt")
                              nc.vector.tensor_mul(yt, yps[h][0:64, :], zr)
                              for k in range(hcho):
                                  nc.sync.dma_start(
                                      out=a2a_ins[bi][hcho * j + k,
                                                      64 * h:64 * h + 64, :],
                                      in_=yt[:, 256 * k:256 * k + 256])
                          else:
                              yt = ynorm.tile([64, 512], F16, tag="yt")
                              nc.vector.tensor_mul(yt, yps[h][0:64, :], zr)
                              for k in range(4):
                                  nc.sync.dma_start(
                                      out=a2a_ins_h[j // 2][4 * (j % 2) + k,
                                                           64 * h:64 * h + 64,
                                                           :],
                                      in_=yt[:, 128 * k:128 * k + 128])
                      # first half-batch collective fires mid-attention
                      if bi == b - 1 and j == tch // 2 - 1:
                          nc.gpsimd.collective_compute(
                              "AllToAll", mybir.AluOpType.bypass,
                              replica_groups=[list(range(ncores))],
                              ins=[a2a_ins_h[0].opt()],
                              outs=[a2a_outs_h[0].opt()],
                          )
                  # resident Wp load: emitted late so it doesn't head-of-line
                  # block the batch-0 x loads; needed only by out-proj 0
                  if wpr is None:
                      wpr = wppool.tile([128, nk, c], F16, tag="wp")
                      nc.sync.dma_start(out=wpr, in_=wpP[:, :, :])
                      wpr_holder["wp"] = wpr
                  if bi < b - 1:
                      nc.gpsimd.collective_compute(
                          "AllToAll", mybir.AluOpType.bypass,
                          replica_groups=[list(range(ncores))],
                          ins=[a2a_ins[bi].opt()], outs=[a2a_outs[bi].opt()],
                      )
                  else:
                      nc.gpsimd.collective_compute(
                          "AllToAll", mybir.AluOpType.bypass,
                          replica_groups=[list(range(ncores))],
                          ins=[a2a_ins_h[1].opt()],
                          outs=[a2a_outs_h[1].opt()],
                      )

                  # out-proj deferred: emitted only once those collectives
                  # are long done, so the yg loads never park at the DMA
                  # queue head blocking later x loads behind it
                  if bi == b - 1:
                      out_proj(list(range(b - 2)))
              out_proj([b - 2])
            # the final rep's tail projections (no next rep to host them)
            out_proj_h(0)
            out_proj_h(1)
    nc.compile()
    return nc


_PROGRAM_CACHE = {}


def _get_program(key=(B, T, C, NCORES)):
    if key not in _PROGRAM_CACHE:
        _PROGRAM_CACHE[key] = build_program(*key)
    return _PROGRAM_CACHE[key]


def _pack(wT, nk=8):
    # [c, o] -> [128, nk, o]: out[p, ck, o] = wT[128*ck + p, o]
    c, o = wT.shape
    return np.ascontiguousarray(
        wT.reshape(nk, 128, o).transpose(1, 0, 2).astype(np.float16))


def make_in_maps(x, Wq, bq, Wk, bk, Wv, bv, Wp, ncores=NCORES):
    bt = x.shape[0] * x.shape[1]
    c = x.shape[2]
    nk = c // 128
    xT = np.ascontiguousarray(x.reshape(bt, c).T.astype(np.float16))
    wpP = _pack(np.asarray(Wp).T.astype(np.float16), nk)
    ident = np.eye(128, dtype=np.float16)
    in_maps = []
    for core in range(ncores):
        s = slice(128 * core, 128 * core + 128)
        in_maps.append({
            "xT": xT,
            "wqP": _pack(Wq[s, :].T.astype(np.float16), nk),
            "wkP": _pack(Wk[s, :].T.astype(np.float16), nk),
            "wvP": _pack(Wv[s, :].T.astype(np.float16), nk),
            "bq": np.ascontiguousarray(bq[s].reshape(128, 1), dtype=np.float32),
            "bk": np.ascontiguousarray(bk[s].reshape(128, 1), dtype=np.float32),
            "bv": np.ascontiguousarray(bv[s].reshape(128, 1), dtype=np.float32),
            "wpP": wpP,
            "ident": ident,
        })
    return in_maps


def assemble_output(results, b=B, t=T, c=C, bp=None):
    bt = b * t
    out = np.empty((bt, c), np.float32)
    for core, res in enumerate(results):
        oT = res["outT"]
        for bi in range(b - 1):
            lo = t * bi + 256 * core
            out[lo:lo + 256, :] = oT[:, 256 * bi:256 * bi + 256].T
        # last batch: two half-batch slices of 128 tokens each
        for hb in range(2):
            lo = t * (b - 1) + (t // 2) * hb + 128 * core
            col = 256 * (b - 1) + 128 * hb
            out[lo:lo + 128, :] = oT[:, col:col + 128].T
    out = out.reshape(b, t, c)
    if bp is not None:
        out = out + bp
    return out


def kernel(x, Wk, bk, Wq, bq, Wv, bv, Wp, bp, _trace=False):
    from concourse.bass_utils import run_bass_kernel_spmd

    x = np.asarray(x, np.float32)
    nc = _get_program()
    in_maps = make_in_maps(x, np.asarray(Wq), np.asarray(bq), np.asarray(Wk),
                           np.asarray(bk), np.asarray(Wv), np.asarray(bv),
                           np.asarray(Wp))
    res = run_bass_kernel_spmd(nc, in_maps, list(range(NCORES)), trace=_trace)
    out = assemble_output(res.results, bp=np.asarray(bp, np.float32))
    if _trace:
        return out, res
    return out



# revision 2
# speedup vs baseline: 1.9775x; 1.9775x over previous
"""Trainium2 Bass kernel for causal self-attention (B=4, T=2048, C=1024, H=16).

Sharding: 2 heads per core across 8 cores (tensor parallel on heads).
v3 = v2 (merged 2-head S PSUM tile + single exp per block, GpSimd causal
zero-fill, direct [token,channel] V projection, bv folded into host bias)
plus software pipelining by EMISSION order: batch bi+1's q/k projection and
V-prep chunks are emitted interleaved inside batch bi's attention loop, so
the Tile scheduler's filler work is adjacent in priority to the attention
stalls it needs to cover, and batch/rep boundaries keep the PE fed.
"""

import numpy as np

import concourse.bass as bass
import concourse.mybir as mybir
import concourse.tile as tile
from concourse import bacc

F32 = mybir.dt.float32
F16 = mybir.dt.float16
EXP = mybir.ActivationFunctionType.Exp
GE = mybir.AluOpType.is_ge

# problem shape (hardcoded per harness contract)
B, T, C, H = 4, 2048, 1024, 16
D = C // H              # 64
NCORES = 8
BT = B * T
TSL = BT // NCORES      # tokens per core after AllToAll
SCALE = 1.0 / np.sqrt(np.float32(D))


def build_program(b=B, t=T, c=C, ncores=NCORES, reps=1):
    """Build the SPMD single-core program. Requires c == 128 * ncores."""
    assert c == 128 * ncores, "2 heads of 64 dims per core"
    bt = b * t
    tsl = bt // ncores
    nk = c // 128            # contraction tiles for projections
    tch = t // 512           # 512-token chunks per batch
    sbk = t // 128           # 128-token s-blocks per batch

    hcho = t // 256 // tch        # half-chunks per 512-chunk (2)
    nc = bacc.Bacc("TRN2", target_bir_lowering=False, num_devices=ncores)

    xT = nc.dram_tensor("xT", [c, bt], F16, kind="ExternalInput")
    wqP = nc.dram_tensor("wqP", [128, nk, 128], F16, kind="ExternalInput")
    wkP = nc.dram_tensor("wkP", [128, nk, 128], F16, kind="ExternalInput")
    wvP = nc.dram_tensor("wvP", [128, nk, 128], F16, kind="ExternalInput")
    bq = nc.dram_tensor("bq", [128, 1], F32, kind="ExternalInput")
    bk = nc.dram_tensor("bk", [128, 1], F32, kind="ExternalInput")
    wpP = nc.dram_tensor("wpP", [128, nk, c], F16, kind="ExternalInput")
    outT = nc.dram_tensor("outT", [c, tsl], F32, kind="ExternalOutput")

    with tile.TileContext(nc) as tc:
        with (
            tc.tile_pool(name="singles", bufs=1) as singles,
            tc.tile_pool(name="dram", bufs=1, space="DRAM") as dram,
            tc.tile_pool(name="xin", bufs=8) as xin,
            tc.tile_pool(name="qkv", bufs=2) as qkv,
            tc.tile_pool(name="vva", bufs=2) as vva,
            tc.tile_pool(name="ptile", bufs=4) as ptile,
            tc.tile_pool(name="ynorm", bufs=3) as ynorm,
            tc.tile_pool(name="wp", bufs=1) as wppool,
            tc.tile_pool(name="outsb", bufs=2) as outsb,
            tc.tile_pool(name="yg", bufs=2) as ygpool,
            tc.tile_pool(name="ps_s", bufs=2, space="PSUM") as ps_s,
            tc.tile_pool(name="ps_y", bufs=1, space="PSUM") as ps_y,
            tc.tile_pool(name="ps_mm", bufs=2, space="PSUM") as ps_mm,
        ):
            a2a_ins = [dram.tile([ncores, 128, 256], F16, name=f"a2ai{i}")
                       for i in range(b - 1)]
            a2a_outs = [dram.tile([ncores, 128, 256], F16, name=f"a2ao{i}")
                        for i in range(b - 1)]
            a2a_ins_h = [dram.tile([ncores, 128, 128], F16, name=f"a2aih{i}")
                         for i in range(2)]
            a2a_outs_h = [dram.tile([ncores, 128, 128], F16, name=f"a2aoh{i}")
                          for i in range(2)]

            w_all = {}
            bias_tiles = {}

            def load_wb(nm, wt, bias, split=False):
                wtile = singles.tile([128, nk, 128], F16, name=f"w{nm}")
                if split:
                    nc.sync.dma_start(out=wtile[:, 0:1, :], in_=wt[:, 0:1, :])
                    nc.sync.dma_start(out=wtile[:, 1:nk, :], in_=wt[:, 1:nk, :])
                else:
                    nc.sync.dma_start(out=wtile, in_=wt[:, :, :])
                w_all[nm] = wtile
                if bias is not None:
                    btile = singles.tile([128, 1], F32, name=f"b{nm}")
                    nc.sync.dma_start(out=btile, in_=bias[:, :])
                    bias_tiles[nm] = btile

            load_wb("q", wqP, bq, split=True)

            def load_rest_singles():
                load_wb("k", wkP, bk)
                load_wb("v", wvP, None)

            wpr_holder = {}

            def out_proj(bis):
                wpr = wpr_holder["wp"]
                nb = len(bis)
                wtok = 256 * nb
                hk = nk // 2
                ygall = ygpool.tile([128, nk, wtok], F16, tag="yg", name="yg")
                for u, bi in enumerate(bis):
                    nc.sync.dma_start(
                        out=ygall[:, :, 256 * u:256 * u + 256],
                        in_=a2a_outs[bi].rearrange("s p g -> p s g"))
                osball = outsb.tile([128, nk, wtok], F32, tag="osb",
                                    name="osb")
                for ph in range(2):
                    for ot in range(hk * ph, hk * ph + hk):
                        ops = ps_mm.tile([128, wtok], F32, tag="mm",
                                         name="ops")
                        for ck in range(nk):
                            nc.tensor.matmul(
                                ops, wpr[:, ck, 128 * ot:128 * ot + 128],
                                ygall[:, ck, :],
                                start=(ck == 0), stop=(ck == nk - 1))
                        nc.vector.tensor_copy(osball[:, ot, :], ops)
                    nc.sync.dma_start(
                        out=outT.rearrange("(ot p) g -> p ot g", p=128)[
                            :, hk * ph:hk * ph + hk,
                            256 * bis[0]:256 * bis[0] + wtok],
                        in_=osball[:, hk * ph:hk * ph + hk, :])

            def load_x(bi, j, split=1):
                t0 = bi * t
                xt = xin.tile([128, nk, 512], F16, tag="xt", name="xt")
                xv = xT.rearrange("(ck p) g -> p ck g", p=128)
                step = nk // split
                for u in range(split):
                    nc.sync.dma_start(
                        out=xt[:, step * u:step * u + step, :],
                        in_=xv[:, step * u:step * u + step,
                               t0 + 512 * j:t0 + 512 * j + 512],
                    )
                return xt

            def out_proj_h(hb):
                wpr = wpr_holder["wp"]
                ygall = ygpool.tile([128, nk, 128], F16, tag="ygh",
                                    name="ygh")
                nc.sync.dma_start(
                    out=ygall,
                    in_=a2a_outs_h[hb].rearrange("s p g -> p s g"))
                osball = outsb.tile([128, nk, 128], F32, tag="osbh",
                                    name="osbh")
                hk = nk // 2
                for ph in range(2):
                    for ot in range(hk * ph, hk * ph + hk):
                        ops = ps_mm.tile([128, 128], F32, tag="mm",
                                         name="ops")
                        for ck in range(nk):
                            nc.tensor.matmul(
                                ops, wpr[:, ck, 128 * ot:128 * ot + 128],
                                ygall[:, ck, :],
                                start=(ck == 0), stop=(ck == nk - 1))
                        nc.vector.tensor_copy(osball[:, ot, :], ops)
                    nc.sync.dma_start(
                        out=outT.rearrange("(ot p) g -> p ot g", p=128)[
                            :, hk * ph:hk * ph + hk,
                            256 * (b - 1) + 128 * hb:
                            256 * (b - 1) + 128 * hb + 128],
                        in_=osball[:, hk * ph:hk * ph + hk, :])

            def emit_proj_chunk(dst_q, dst_k, xt, j):
                for nm, dst in (("q", dst_q), ("k", dst_k)):
                    ps = ps_mm.tile([128, 512], F32, tag="mm")
                    for ck in range(nk):
                        nc.tensor.matmul(
                            ps, w_all[nm][:, ck, :], xt[:, ck, :],
                            start=(ck == 0), stop=(ck == nk - 1),
                        )
                    nc.vector.tensor_scalar_add(
                        dst[:, 512 * j:512 * j + 512], ps, bias_tiles[nm])

            def emit_vprep_blocks(vv, xts_local, blocks):
                # V directly in [token, channel] layout: x chunk stationary,
                # Wv streaming; vv = [v0 | 1s | v1 | 1s] per s-block
                for i in blocks:
                    xt = xts_local[i // 4]
                    o = 128 * (i % 4)
                    vps = ps_mm.tile([128, 128], F32, tag="mm")
                    for ck in range(nk):
                        nc.tensor.matmul(
                            vps, xt[:, ck, o:o + 128],
                            w_all["v"][:, ck, :],
                            start=(ck == 0), stop=(ck == nk - 1))
                    nc.vector.tensor_copy(
                        vv[:, i, 0::2, :],
                        vps.rearrange("p (s g) -> p s g", s=2))

            def alloc_batch_tiles():
                qT = qkv.tile([128, t], F16, tag="qT")
                kT = qkv.tile([128, t], F16, tag="kT")
                vv = vva.tile([128, sbk, 4, 64], F16, tag="vv")
                nc.vector.memset(vv[:, :, 1, :], 1.0)
                nc.vector.memset(vv[:, :, 3, :], 1.0)
                return qT, kT, vv

            def emit_attention_chunk(bi, j, qT, kT, vv):
                yps = []
                for h in range(2):
                    yp = ps_y.tile([128, 512], F32, tag=f"yp{h}")
                    yps.append(yp)
                nsb = 4 * j + 4
                for i in range(nsb):
                    toff = max(0, 128 * i - 512 * j)
                    w = 512 - toff
                    sp = ps_s.tile([128, 2, 512], F32, tag="sp")
                    for h in range(2):
                        d0 = 64 * h
                        nc.tensor.matmul(
                            sp[:, h, :w],
                            kT[d0:d0 + 64, 128 * i:128 * i + 128],
                            qT[d0:d0 + 64, 512 * j + toff:512 * j + 512],
                            start=True, stop=True,
                        )
                    pt = ptile.tile([128, 2, 512], F16, tag="pt")
                    nc.scalar.activation(
                        pt[:, :, :w], sp[:, :, :w], EXP, scale=float(SCALE))
                    if 128 * i >= 512 * j:
                        nc.gpsimd.affine_select(
                            out=pt[:, :, 0:128], in_=pt[:, :, 0:128],
                            compare_op=GE, fill=0.0, base=0,
                            channel_multiplier=-1,
                            pattern=[[0, 2], [1, 128]],
                        )
                    for h in range(2):
                        nc.tensor.matmul(
                            yps[h][:, toff:512],
                            vv[:, i, 2 * h:2 * h + 2, :],
                            pt[:, h, :w],
                            start=(i == 0), stop=(i == nsb - 1),
                            skip_group_check=True,
                        )
                for h in range(2):
                    zr = ynorm.tile([64, 512], F32, tag="zr")
                    nc.vector.reciprocal(zr, yps[h][64:128, :])
                    yt = ynorm.tile([64, 512], F16, tag="yt")
                    nc.vector.tensor_mul(yt, yps[h][0:64, :], zr)
                    if bi < b - 1:
                        for k in range(hcho):
                            nc.sync.dma_start(
                                out=a2a_ins[bi][hcho * j + k,
                                                64 * h:64 * h + 64, :],
                                in_=yt[:, 256 * k:256 * k + 256])
                    else:
                        for k in range(4):
                            nc.sync.dma_start(
                                out=a2a_ins_h[j // 2][4 * (j % 2) + k,
                                                     64 * h:64 * h + 64, :],
                                in_=yt[:, 128 * k:128 * k + 128])
                if bi == b - 1 and j == tch // 2 - 1:
                    nc.gpsimd.collective_compute(
                        "AllToAll", mybir.AluOpType.bypass,
                        replica_groups=[list(range(ncores))],
                        ins=[a2a_ins_h[0].opt()],
                        outs=[a2a_outs_h[0].opt()],
                    )

            # ---------------- main schedule ----------------
            cur = None          # (qT, kT, vv) of the batch about to attend
            nxt_xts = None      # x tiles of the next batch (for its proj)
            for _rep in range(reps):
              wpr = None
              if cur is None:
                  # prologue: load + fully project batch 0
                  xts0 = [load_x(0, 0, split=2)]
                  load_rest_singles()
                  xts0 += [load_x(0, j) for j in range(1, tch)]
                  cur = alloc_batch_tiles()
                  for j in range(tch):
                      emit_proj_chunk(cur[0], cur[1], xts0[j], j)
                  emit_vprep_blocks(cur[2], xts0, range(sbk))
              for bi in range(b):
                  if bi + 1 < b:
                      have_next = True
                  elif _rep + 1 < reps:
                      have_next = True
                  else:
                      have_next = False
                  if have_next:
                      nb = bi + 1 if bi + 1 < b else 0
                      nxt_xts = [load_x(nb, j) for j in range(tch)]
                      nxt = alloc_batch_tiles()
                  for j in range(tch):
                      emit_attention_chunk(bi, j, *cur)
                      if have_next:
                          emit_proj_chunk(nxt[0], nxt[1], nxt_xts[j], j)
                          emit_vprep_blocks(nxt[2], nxt_xts,
                                            range(4 * j, 4 * j + 4))
                  # previous rep's tail out-projections: hosted after batch
                  # 0's attention so they fill the half-collective window
                  if bi == 0 and _rep > 0:
                      out_proj_h(0)
                      out_proj_h(1)
                  if wpr is None:
                      wpr = wppool.tile([128, nk, c], F16, tag="wp")
                      nc.sync.dma_start(out=wpr, in_=wpP[:, :, :])
                      wpr_holder["wp"] = wpr
                  if bi < b - 1:
                      nc.gpsimd.collective_compute(
                          "AllToAll", mybir.AluOpType.bypass,
                          replica_groups=[list(range(ncores))],
                          ins=[a2a_ins[bi].opt()], outs=[a2a_outs[bi].opt()],
                      )
                  else:
                      nc.gpsimd.collective_compute(
                          "AllToAll", mybir.AluOpType.bypass,
                          replica_groups=[list(range(ncores))],
                          ins=[a2a_ins_h[1].opt()],
                          outs=[a2a_outs_h[1].opt()],
                      )
                  if have_next:
                      cur = nxt
                  if bi == b - 1:
                      out_proj(list(range(b - 2)))
              out_proj([b - 2])
            out_proj_h(0)
            out_proj_h(1)
    nc.compile()
    return nc


_PROGRAM_CACHE = {}


def _get_program(key=(B, T, C, NCORES)):
    if key not in _PROGRAM_CACHE:
        _PROGRAM_CACHE[key] = build_program(*key)
    return _PROGRAM_CACHE[key]


def _pack(wT, nk=8):
    # [c, o] -> [128, nk, o]: out[p, ck, o] = wT[128*ck + p, o]
    c, o = wT.shape
    return np.ascontiguousarray(
        wT.reshape(nk, 128, o).transpose(1, 0, 2).astype(np.float16))


def make_in_maps(x, Wq, bq, Wk, bk, Wv, bv, Wp, ncores=NCORES):
    bt = x.shape[0] * x.shape[1]
    c = x.shape[2]
    nk = c // 128
    xT = np.ascontiguousarray(x.reshape(bt, c).T.astype(np.float16))
    wpP = _pack(np.asarray(Wp).T.astype(np.float16), nk)
    in_maps = []
    for core in range(ncores):
        s = slice(128 * core, 128 * core + 128)
        in_maps.append({
            "xT": xT,
            "wqP": _pack(Wq[s, :].T.astype(np.float16), nk),
            "wkP": _pack(Wk[s, :].T.astype(np.float16), nk),
            "wvP": _pack(Wv[s, :].T.astype(np.float16), nk),
            "bq": np.ascontiguousarray(bq[s].reshape(128, 1), dtype=np.float32),
            "bk": np.ascontiguousarray(bk[s].reshape(128, 1), dtype=np.float32),
            "wpP": wpP,
        })
    return in_maps


def assemble_output(results, b=B, t=T, c=C, bp=None):
    bt = b * t
    out = np.empty((bt, c), np.float32)
    for core, res in enumerate(results):
        oT = res["outT"]
        for bi in range(b - 1):
            lo = t * bi + 256 * core
            out[lo:lo + 256, :] = oT[:, 256 * bi:256 * bi + 256].T
        for hb in range(2):
            lo = t * (b - 1) + (t // 2) * hb + 128 * core
            col = 256 * (b - 1) + 128 * hb
            out[lo:lo + 128, :] = oT[:, col:col + 128].T
    out = out.reshape(b, t, c)
    if bp is not None:
        out = out + bp
    return out


def kernel(x, Wk, bk, Wq, bq, Wv, bv, Wp, bp, _trace=False):
    from concourse.bass_utils import run_bass_kernel_spmd

    x = np.asarray(x, np.float32)
    nc = _get_program()
    in_maps = make_in_maps(x, np.asarray(Wq), np.asarray(bq), np.asarray(Wk),
                           np.asarray(bk), np.asarray(Wv), np.asarray(bv),
                           np.asarray(Wp))
    res = run_bass_kernel_spmd(nc, in_maps, list(range(NCORES)), trace=_trace)
    # bv folds into the output bias: softmax rows sum to 1, so attention
    # adds exactly bv per channel; out += Wp @ bv + bp
    bp_eff = (np.asarray(bp, np.float32)
              + np.asarray(Wp, np.float32) @ np.asarray(bv, np.float32))
    out = assemble_output(res.results, bp=bp_eff)
    if _trace:
        return out, res
    return out
